# revision 37
# baseline (speedup 1.0000x reference)
"""DeepseekV3 top-k router (moe_routing) on 8 Trainium2 NeuronCores.

Sharding (hardcoded from the problem spec):
  - Data-parallel over the token dim: 8192 tokens -> 8 shards of 1024.
  - Router weight [256, 7168] and bias [256] replicated to every core.

Layout/precision prep on host (inside kernel()):
  - Each x shard is fed transposed and pre-tiled to the exact SBUF layout
    [NB, 128, KT, 256] so every DMA moves long contiguous per-partition runs.
  - fp32 operands are split into fp16 pairs: v = v_hi + 2^-11 * v_lo with
    v_hi = fp16(v), v_lo = fp16((v - v_hi) * 2^11). The device computes
    logits = x_hi.w_hi + 2^-11 * (x_hi.w_lo + x_lo.w_hi), which matches
    fp32 matmul precision (verified: error vs fp64 identical to fp32) while
    running the PE at 1 cycle/row instead of fp32's 4.
  - w_hi and w_lo are interleaved per k-tile ([P, KT, 2, E]) so the hh and
    hl terms run as ONE N=512 matmul per k (fewer weight loads).

Per-core device kernel (Bass/Tile):
  - Warm-up matmuls from t=0 keep the PE HAM clock-gate at 2.4 GHz.
  - Per 256-token block: load xT hi/lo slices, then per 128-token tile
    accumulate hh into PSUM[:, :256] and the cross terms into PSUM[:, 256:]
    over 56 k-tiles (2 matmuls per k); combine on ScalarE+VectorE; sigmoid
    on ScalarE; then the DeepseekV3 grouped top-k epilogue on VectorE
    (group top-2 via segmented max + match_replace, top-4 groups via sort8 +
    threshold, top-8 experts via max/max_index, score gather via fused
    scalar_tensor_tensor (iota == idx) * s with accumulate).

Measured on 8 axon-tunneled trn2 cores: ~193-204 us HW exec (NTFF, core 0),
0/65536 index mismatches vs the fp32 jax reference, weight rel err 7.6e-7.
"""

import os
import sys

for _p in ("/opt/trn_rl_repo", "/root/.axon_site/_ro/trn_rl_repo"):
    if os.path.isdir(_p) and _p not in sys.path:
        sys.path.append(_p)

from contextlib import ExitStack

import numpy as np

import concourse.bass as bass
import concourse.bacc as bacc
import concourse.mybir as mybir
import concourse.tile as tile

N_CORES = 8
T_FULL = 8192
HIDDEN = 7168
N_EXPERTS = 256
TOP_K = 8
N_GROUP = 8
TOPK_GROUP = 4
SCALING = 2.5

P = 128
TB = 256                      # tokens per DMA block (2 tiles)
LO_SCALE = 2.0 ** 11
F32 = mybir.dt.float32
F16 = mybir.dt.float16
WARMUP_MMS = 58
WCH = 4                       # k-chunks for streaming w / first x block


def build_module(t_shard=T_FULL // N_CORES, hidden=HIDDEN):
    """Build + compile the per-core Bass module (SPMD: same program, 8 cores)."""
    KT = hidden // P            # hidden k-tiles (56)
    TT = t_shard // P           # token tiles per core (8)
    NB = t_shard // TB          # token DMA blocks (4)
    E = N_EXPERTS
    EPG = E // N_GROUP          # experts per group (32)
    AX = mybir.AxisListType
    OP = mybir.AluOpType

    nc = bacc.Bacc("TRN2", debug=False, target_bir_lowering=False)

    # pre-tiled inputs (see _make_in_maps)
    xh = nc.dram_tensor("xT_hi", [NB, P, KT, TB], F16, kind="ExternalInput").ap()
    xl = nc.dram_tensor("xT_lo", [NB, P, KT, TB], F16, kind="ExternalInput").ap()
    whl = nc.dram_tensor("wT_hilo", [P, KT, 2, E], F16, kind="ExternalInput").ap()
    bias = nc.dram_tensor("bias", [E], F32, kind="ExternalInput").ap()
    out_i = nc.dram_tensor("topk_idx", [t_shard, TOP_K], mybir.dt.int32,
                           kind="ExternalOutput").ap()
    out_w = nc.dram_tensor("topk_w", [t_shard, TOP_K], F32,
                           kind="ExternalOutput").ap()
    sink = nc.dram_tensor("warm_sink", [P, 1], F32).ap()

    # uneven chunks: small first chunk so the first matmuls start early,
    # larger later chunks for DMA efficiency (eighths: 1/8, 2/8, 2/8, 3/8)
    cuts = [0, max(1, KT // 8), max(2, (3 * KT) // 8), max(3, (5 * KT) // 8), KT]
    kranges = [(cuts[i], cuts[i + 1]) for i in range(4) if cuts[i] < cuts[i + 1]]

    with tile.TileContext(nc) as tc, ExitStack() as ctx:
        const = ctx.enter_context(tc.tile_pool(name="const", bufs=1))
        wpool = ctx.enter_context(tc.tile_pool(name="wres", bufs=1))
        xpool = ctx.enter_context(tc.tile_pool(name="xin", bufs=2))
        spool = ctx.enter_context(tc.tile_pool(name="scr", bufs=2))
        smalls = ctx.enter_context(tc.tile_pool(name="small", bufs=2))
        opool = ctx.enter_context(tc.tile_pool(name="outs", bufs=1))
        pspool = ctx.enter_context(tc.tile_pool(name="ps", bufs=2, space="PSUM"))
        pswarm = ctx.enter_context(tc.tile_pool(name="psw", bufs=1, space="PSUM"))

        # ---- PE warm-up: keep the HAM clock-gate busy from t=0 ----
        wu = const.tile([P, E], F16)
        nc.gpsimd.memset(wu[:], 0.0)
        psw = pswarm.tile([P, E], F32)
        for _ in range(WARMUP_MMS):
            nc.tensor.matmul(psw[:], wu[:, :P], wu[:], start=True, stop=True)
        wsum = smalls.tile([P, 1], F32, tag="wsum")
        nc.vector.tensor_reduce(wsum[:], psw[:], axis=AX.X, op=OP.add)
        # SWDGE ring: must not block the HWDGE rings while warmup runs
        nc.gpsimd.dma_start(out=sink, in_=wsum[:])

        # ---- constants (bias is emitted after the first w chunks) ----
        bias_bc = const.tile([P, E], F32)
        bias_src = bass.AP(tensor=bias.tensor, offset=0, ap=[[0, P], [1, E]])

        iota_f = const.tile([P, E], F32)
        nc.gpsimd.iota(iota_f[:], pattern=[[1, E]], base=0, channel_multiplier=0,
                       allow_small_or_imprecise_dtypes=True)

        # ---- resident w hi|lo interleaved [P, KT, 2, E] ----
        w_sb = wpool.tile([P, KT, 2, E], F16)

        out_i_sb = opool.tile([P, TT, TOP_K], mybir.dt.int32)
        out_w_sb = opool.tile([P, TT, TOP_K], F32)

        def emit_mms(ps, xt_hi, xt_lo, tsl, k):
            # hh into ps[:, :256] and hl into ps[:, 256:] in one N=512 MM;
            # lh accumulates into ps[:, 256:]. On the last k the wide MM goes
            # last with stop=True so the whole bank's accumulation group
            # closes with the final matmul.
            wide = (ps[:], xt_hi[:, k, tsl],
                    w_sb[:, k].rearrange("p a e -> p (a e)"))
            narrow = (ps[:, E:], xt_lo[:, k, tsl], w_sb[:, k, 0, :])
            if k < KT - 1:
                nc.tensor.matmul(*wide, start=(k == 0), stop=False)
                nc.tensor.matmul(*narrow, start=False, stop=False)
            else:
                nc.tensor.matmul(*narrow, start=False, stop=False)
                nc.tensor.matmul(*wide, start=False, stop=True)

        def epilogue(tt, ps):
            # logits = ps[:, :256] + 2^-11 * ps[:, 256:]
            sA = spool.tile([P, E], F32, tag="sA")
            nc.scalar.activation(sA[:], ps[:, :E],
                                 mybir.ActivationFunctionType.Copy)
            comb = spool.tile([P, E], F32, tag="comb")
            nc.vector.scalar_tensor_tensor(comb[:], ps[:, E:], 1.0 / LO_SCALE,
                                           sA[:], op0=OP.mult, op1=OP.add)

            s = spool.tile([P, E], F32, tag="s")
            nc.scalar.activation(s[:], comb[:],
                                 mybir.ActivationFunctionType.Sigmoid)

            # scores for choice = sigmoid + bias
            sc = spool.tile([P, E], F32, tag="sc")
            nc.vector.tensor_tensor(sc[:], s[:], bias_bc[:], op=OP.add)

            sc_g = sc[:].rearrange("p (g c) -> p g c", c=EPG)

            # per-group top-2 sum
            gmax = smalls.tile([P, N_GROUP], F32, tag="gmax")
            nc.vector.tensor_reduce(gmax[:], sc_g, axis=AX.X, op=OP.max)
            rep = spool.tile([P, E], F32, tag="rep")
            nc.vector.match_replace(rep[:], gmax[:], sc[:], -1e30)
            gsec = smalls.tile([P, N_GROUP], F32, tag="gsec")
            nc.vector.tensor_reduce(gsec[:],
                                    rep[:].rearrange("p (g c) -> p g c", c=EPG),
                                    axis=AX.X, op=OP.max)
            gsum = smalls.tile([P, N_GROUP], F32, tag="gsum")
            nc.vector.tensor_tensor(gsum[:], gmax[:], gsec[:], op=OP.add)

            # top-4 groups: sort the 8 group scores, threshold at 4th
            gsort = smalls.tile([P, 8], F32, tag="gsort")
            nc.vector.max(gsort[:], gsum[:])
            gmask = smalls.tile([P, N_GROUP], F32, tag="gmask")
            nc.vector.tensor_scalar(gmask[:], gsum[:],
                                    gsort[:, TOPK_GROUP - 1:TOPK_GROUP], None,
                                    op0=OP.is_ge)

            # masked scores = sc * group_mask
            masked = spool.tile([P, E], F32, tag="masked")
            nc.vector.tensor_tensor(masked[:].rearrange("p (g c) -> p g c", c=EPG),
                                    sc_g,
                                    gmask[:].unsqueeze(2).broadcast_to(
                                        (P, N_GROUP, EPG)),
                                    op=OP.mult)

            # top-8 experts (desc values + indices, lax.top_k semantics)
            t8v = smalls.tile([P, TOP_K], F32, tag="t8v")
            nc.vector.max(t8v[:], masked[:])
            t8i = smalls.tile([P, TOP_K], mybir.dt.uint32, tag="t8i")
            nc.vector.max_index(t8i[:], t8v[:], masked[:])

            # output copy rides GpSimd so it stays off the DVE chain
            nc.gpsimd.tensor_copy(out_i_sb[:, tt, :], t8i[:])

            # gather sigmoid scores at the top-8 indices without per-k scans:
            # mark the 8 selected positions (match_replace diff), extract the
            # selected s values sorted by s (max/max_index), then permute to
            # choice order with an 8x8 index match.
            mr2 = spool.tile([P, E], F32, tag="mr2")
            nc.vector.match_replace(mr2[:], t8v[:], masked[:], -1.0)
            sel = spool.tile([P, E], F32, tag="sel")
            nc.vector.tensor_tensor(sel[:], mr2[:], masked[:], op=OP.not_equal)
            nc.vector.tensor_tensor(sel[:], sel[:], s[:], op=OP.mult)
            v8 = smalls.tile([P, TOP_K], F32, tag="v8")
            nc.vector.max(v8[:], sel[:])
            i8 = smalls.tile([P, TOP_K], mybir.dt.uint32, tag="i8")
            nc.vector.max_index(i8[:], v8[:], sel[:])
            # eqm[p, k, j] = (idx_choice[p, k] == idx_s[p, j]); sg = eqm @ v8
            # (compare the uint32 indices directly, f32 0/1 out)
            eqm = smalls.tile([P, TOP_K, TOP_K], F32, tag="eqm")
            nc.vector.tensor_tensor(eqm[:],
                                    t8i[:].unsqueeze(2).broadcast_to(
                                        (P, TOP_K, TOP_K)),
                                    i8[:].unsqueeze(1).broadcast_to(
                                        (P, TOP_K, TOP_K)),
                                    op=OP.is_equal)
            nc.vector.tensor_tensor(eqm[:], eqm[:],
                                    v8[:].unsqueeze(1).broadcast_to(
                                        (P, TOP_K, TOP_K)),
                                    op=OP.mult)
            sg = smalls.tile([P, TOP_K], F32, tag="sg")
            nc.vector.tensor_reduce(sg[:], eqm[:], axis=AX.X, op=OP.add)

            # weights = sg / sum(sg) * SCALING
            den = smalls.tile([P, 1], F32, tag="den")
            nc.vector.tensor_reduce(den[:], sg[:], axis=AX.X, op=OP.add)
            rcp = smalls.tile([P, 1], F32, tag="rcp")
            nc.vector.reciprocal(rcp[:], den[:])
            nc.vector.tensor_scalar(out_w_sb[:, tt, :], sg[:], rcp[:, 0:1],
                                    SCALING, op0=OP.mult, op1=OP.mult)

        nsub = TB // P
        oi = out_i.rearrange("(t p) k -> p t k", p=P)
        ow = out_w.rearrange("(t p) k -> p t k", p=P)

        for tb in range(NB):
            xt_hi = xpool.tile([P, KT, TB], F16, tag="xth", name=f"xth_{tb}")
            xt_lo = xpool.tile([P, KT, TB], F16, tag="xtl", name=f"xtl_{tb}")
            if tb == 0:
                # chunked arrival so the first matmuls start early; balance
                # both HWDGE rings: sync gets x_hi + first w half, scalar
                # gets x_lo + second w half
                for c, (k0, k1) in enumerate(kranges):
                    km = (k0 + k1) // 2
                    nc.sync.dma_start(out=xt_hi[:, k0:k1], in_=xh[tb, :, k0:k1])
                    nc.scalar.dma_start(out=xt_lo[:, k0:k1], in_=xl[tb, :, k0:k1])
                    if km > k0:
                        nc.sync.dma_start(out=w_sb[:, k0:km], in_=whl[:, k0:km])
                    nc.scalar.dma_start(out=w_sb[:, km:k1], in_=whl[:, km:k1])
                nc.scalar.dma_start(out=bias_bc[:], in_=bias_src)
            else:
                nc.sync.dma_start(out=xt_hi[:], in_=xh[tb])
                nc.scalar.dma_start(out=xt_lo[:], in_=xl[tb])

            if tb == 0:
                # DMA-starved phase: interleave both sub-tiles in one k-loop
                # so every arriving k-chunk feeds 4 matmuls immediately
                pss = []
                for s in range(nsub):
                    ps_s = pspool.tile([P, 2 * E], F32, tag=f"ps{s}", name=f"ps_{s}")
                    pss.append(ps_s)
                for k in range(KT):
                    for s in range(nsub):
                        emit_mms(pss[s], xt_hi, xt_lo,
                                 slice(s * P, (s + 1) * P), k)
                for s in range(nsub):
                    epilogue(tb * nsub + s, pss[s])
            else:
                for s in range(nsub):
                    ps = pspool.tile([P, 2 * E], F32, tag=f"ps{s}")
                    for k in range(KT):
                        emit_mms(ps, xt_hi, xt_lo,
                                 slice(s * P, (s + 1) * P), k)
                    epilogue(tb * nsub + s, ps)

            # outputs for this block: token t = tt*P + p
            t0 = tb * nsub
            nc.scalar.dma_start(out=oi[:, t0:t0 + nsub],
                                in_=out_i_sb[:, t0:t0 + nsub])
            nc.scalar.dma_start(out=ow[:, t0:t0 + nsub],
                                in_=out_w_sb[:, t0:t0 + nsub])

    nc.compile()
    return nc


_CACHED = {}


def _get_module():
    key = (T_FULL // N_CORES, HIDDEN)
    if key not in _CACHED:
        _CACHED[key] = build_module(*key)
    return _CACHED[key]


def _split_f16(a):
    hi = a.astype(np.float16)
    lo = ((a - hi.astype(np.float32)) * np.float32(LO_SCALE)).astype(np.float16)
    return hi, lo


def _tile_x(shardT, t_shard, hidden):
    # [H, T] -> [NB, P, KT, TB]   (h = k*P + p, t = nb*TB + c)
    KT = hidden // P
    NB = t_shard // TB
    v = shardT.reshape(KT, P, NB, TB)
    return np.ascontiguousarray(v.transpose(2, 1, 0, 3))


def _tile_w(wT_hi, wT_lo, hidden):
    # two [H, E] -> [P, KT, 2, E]
    KT = hidden // P
    E = wT_hi.shape[1]
    out = np.empty((P, KT, 2, E), dtype=np.float16)
    out[:, :, 0, :] = wT_hi.reshape(KT, P, E).transpose(1, 0, 2)
    out[:, :, 1, :] = wT_lo.reshape(KT, P, E).transpose(1, 0, 2)
    return np.ascontiguousarray(out)


def _make_in_maps(x, weight, e_score_correction_bias):
    x = np.asarray(x, dtype=np.float32)
    w = np.asarray(weight, dtype=np.float32)
    b = np.ascontiguousarray(np.asarray(e_score_correction_bias, dtype=np.float32))
    hidden = x.shape[1]
    wT = np.ascontiguousarray(w.T)
    wT_hi, wT_lo = _split_f16(wT)
    w_hilo = _tile_w(wT_hi, wT_lo, hidden)
    t_shard = x.shape[0] // N_CORES
    in_maps = []
    for i in range(N_CORES):
        shard = np.ascontiguousarray(x[i * t_shard:(i + 1) * t_shard].T)
        xT_hi, xT_lo = _split_f16(shard)
        in_maps.append({"xT_hi": _tile_x(xT_hi, t_shard, hidden),
                        "xT_lo": _tile_x(xT_lo, t_shard, hidden),
                        "wT_hilo": w_hilo, "bias": b})
    return in_maps


def run_hw(x, weight, e_score_correction_bias, trace=False, **kwargs):
    """Run on the 8 NeuronCores; returns ((idx, w), BassKernelResults)."""
    from concourse.bass_utils import run_bass_kernel_spmd

    nc = _get_module()
    in_maps = _make_in_maps(x, weight, e_score_correction_bias)
    res = run_bass_kernel_spmd(nc, in_maps, core_ids=list(range(N_CORES)),
                               trace=trace, **kwargs)
    idx = np.concatenate([r["topk_idx"] for r in res.results], axis=0)
    w = np.concatenate([r["topk_w"] for r in res.results], axis=0)
    return (idx.astype(np.int32, copy=False), w.astype(np.float32, copy=False)), res


def kernel(x, weight, e_score_correction_bias):
    (idx, w), _ = run_hw(x, weight, e_score_correction_bias, trace=False)
    return idx, w
